# revision 16
# baseline (speedup 1.0000x reference)
"""Trainium2 Bass kernel for nn_BlockSparseMLP (MoE gated MLP, E=8, top-2).

Strategy: expert parallelism over 8 NeuronCores. The router matmul
(x @ w_router, 67 MFLOP out of the 206 GFLOP total) plus the top-2
dispatch/gather and the final scatter-add combine run on the host; each
core runs the full gated MLP (gate/up, silu*up, down, weighted by the
routing prob) for the tokens routed to its expert.

All matmul operands are bf16 (host-converted; PSUM accumulation stays
fp32): full PE rate (1 row/cycle) like fp32r, half the HBM traffic.
Weights are streamed as contiguous 1 MB transfers ([128 part, 8 k-tiles,
512] halves, packed on the host so partition-major rows are 8 KB runs) —
128 KB tile-at-a-time DMA only sustains ~75 GB/s/queue and starved the
PE at startup; 1 MB transfers run at ~340 GB/s and keep the weight
stream ahead of the PE for the whole kernel.

Per-core device layout (capacity C = 512 tokens):
  phase 1 (gate/up): per I-chunk of 256, two 1 MB weight halves
    (gate|up packed side by side); weights stationary, xT moving
    (N=512). silu(gate)*up fused on ACT+DVE into aT ([I, C] bf16,
    SBUF-resident).
  phase 2 (down): stream w_down as 1 MB halves (moving [128,512]
    slices), aT tiles stationary, accumulate over I into [tokens, 512]
    psum tiles, scale by the routing weight on DVE, contiguous 256 KB
    fp32 stores on the HWDGE queues.
"""

import sys
import functools

sys.path.insert(0, "/opt/trn_rl_repo")

import numpy as np
import ml_dtypes

BF16 = ml_dtypes.bfloat16

T, H, II, E, TOPK = 2048, 2048, 4096, 8, 2
NCORES = 8
B0 = 512        # per-expert token capacity (moving N)
CHUNK = 256     # phase-1 I-chunk width
KT = H // 128   # 16 contraction tiles for gate/up
MTI = II // 128  # 32 I tiles
NMC = II // CHUNK  # 16 phase-1 chunks
JJ = CHUNK // 128  # 2 m-tiles per chunk
KI = II // 128  # 32 contraction tiles for down
NH = H // 512   # 4 output column chunks
NT = B0 // 128  # 4 token tiles
XQ = 4          # x is loaded as 4 quarter tiles of 4 k-slices each
WHK = 8         # k-tiles per 1 MB weight half


@functools.lru_cache(maxsize=2)
def _build(nb1: int = 0):
    """Build the SPMD Bass program (capacity B0 tokens; nb1 kept for
    test.py signature compat and must be 0 — overflow spills to host)."""
    assert nb1 == 0
    import concourse.mybir as mybir
    import concourse.tile as tile
    from concourse import bacc

    f32 = mybir.dt.float32
    bf16 = mybir.dt.bfloat16

    nc = bacc.Bacc(None)
    xT0 = nc.declare_dram_parameter("xT0", [128, KT, B0], bf16, isOutput=False)
    wgu = nc.declare_dram_parameter("wgu", [NMC, 2, 128, WHK, 2 * CHUNK], bf16, isOutput=False)
    wd = nc.declare_dram_parameter("wd", [NH, KI // WHK, 128, WHK, 512], bf16, isOutput=False)
    rw = nc.declare_dram_parameter("rw", [128, NT], f32, isOutput=False)
    dout = nc.declare_dram_parameter("d", [NH, NT, 128, 512], f32, isOutput=True)

    SILU = mybir.ActivationFunctionType.Silu

    with tile.TileContext(nc) as tc:
        with (
            tc.tile_pool(name="pers", bufs=1) as pers,
            tc.tile_pool(name="wpool", bufs=8) as wpool,
            tc.tile_pool(name="wdpool", bufs=4) as wdpool,
        ):
            aT0 = pers.tile([128, MTI, B0], bf16)
            rwt = pers.tile([128, NT], f32)

            with (
                tc.tile_pool(name="xp", bufs=1) as xp,
                tc.tile_pool(name="ps1", bufs=1, space="PSUM") as ps1,
                tc.tile_pool(name="sp", bufs=2) as sp,
            ):
                # Startup choreography: DMA delivery crawls for the first
                # ~5 us (cold HBM + all 8 cores bursting at once), so the
                # tiles consumed first are the smallest — x and chunk-0
                # weights ramp 128-512 KB. x rides sync/gpsimd, chunk-0
                # weights scalar/sync; later chunks stream as 1 MB halves
                # alternating scalar/sync.
                xs = []          # (tile, nk) covering k = base..base+nk
                for base, nk, eng in ((0, 2, nc.sync), (2, 2, nc.sync),
                                      (4, 4, nc.gpsimd), (8, 4, nc.gpsimd),
                                      (12, 4, nc.gpsimd)):
                    xt = xp.tile([128, nk, B0], bf16, name=f"xs{base}")
                    eng.dma_start(xt[:], xT0[:, base:base + nk, :])
                    xs.append((base, nk, xt))
                nc.gpsimd.dma_start(rwt[:], rw[:])

                def xk_of(k):
                    for base, nk, xt in xs:
                        if base <= k < base + nk:
                            return xt[:, k - base, :]
                    raise AssertionError

                for mc in range(NMC):
                    if mc == 0:
                        wq = []  # (base, nk, tile)
                        for i, (base, nk) in enumerate(
                                ((0, 1), (1, 1), (2, 1), (3, 1),
                                 (4, 4), (8, 4), (12, 4))):
                            wt = xp.tile([128, nk, 2 * CHUNK], bf16, name=f"wq{i}")
                            eng = nc.scalar if i % 2 == 0 else nc.sync
                            h, o = divmod(base, WHK)
                            eng.dma_start(wt[:], wgu[0, h][:, o:o + nk, :])
                            wq.append((base, nk, wt))

                        def wk_of(k):
                            for base, nk, wt in wq:
                                if base <= k < base + nk:
                                    return wt[:, k - base, :]
                            raise AssertionError
                    else:
                        wb = []
                        for h in range(2):
                            hidx = mc * 2 + h
                            eng = nc.scalar if hidx % 2 == 0 else nc.sync
                            wbh = wpool.tile([128, WHK, 2 * CHUNK], bf16, name="wbh", tag="w")
                            eng.dma_start(wbh[:], wgu[mc, h])
                            wb.append(wbh)
                        wk_of = lambda k, wb=wb: wb[k // WHK][:, k % WHK, :]

                    pg0 = [ps1.tile([128, B0], f32, name="pg0", tag="pg0", bufs=3)
                           for _ in range(JJ)]
                    pu0 = [ps1.tile([128, B0], f32, name="pu0", tag="pu0", bufs=3)
                           for _ in range(JJ)]

                    for k in range(KT):
                        st = dict(start=(k == 0), stop=(k == KT - 1))
                        wk = wk_of(k)
                        xk = xk_of(k)
                        for j in range(JJ):
                            nc.tensor.matmul(
                                pg0[j][:], wk[:, j * 128:(j + 1) * 128], xk, **st
                            )
                        for j in range(JJ):
                            nc.tensor.matmul(
                                pu0[j][:],
                                wk[:, CHUNK + j * 128:CHUNK + (j + 1) * 128],
                                xk, **st
                            )

                    for j in range(JJ):
                        m = mc * JJ + j
                        sg = sp.tile([128, B0], f32, name="sg", tag="sg")
                        nc.scalar.activation(sg[:], pg0[j][:], SILU)
                        nc.vector.tensor_mul(aT0[:, m, :], sg[:], pu0[j][:])

            with (
                tc.tile_pool(name="ps2", bufs=1, space="PSUM") as ps2,
                tc.tile_pool(name="op", bufs=6) as op,
            ):
                for nh in range(NH):
                    pd = [ps2.tile([128, 512], f32, name="pd", tag="pd", bufs=7)
                          for _ in range(NT)]
                    for h in range(KI // WHK):
                        hidx = nh * (KI // WHK) + h
                        # dedicated pool: the first 4 halves have no slot
                        # dependency on phase-1 tiles, so they prefetch during
                        # phase 1 and the phase transition never stalls. sync
                        # leads — scalar is busy with the last silu/mul work.
                        eng = nc.sync if hidx % 2 == 0 else nc.scalar
                        wdh = wdpool.tile([128, WHK, 512], bf16, name="wdh", tag="wd")
                        eng.dma_start(wdh[:], wd[nh, h])
                        for kk in range(WHK):
                            k = h * WHK + kk
                            st = dict(start=(k == 0), stop=(k == KI - 1))
                            for mt in range(NT):
                                nc.tensor.matmul(
                                    pd[mt][:], aT0[:, k, mt * 128:(mt + 1) * 128],
                                    wdh[:, kk, :], **st
                                )
                    for mt in range(NT):
                        ot = op.tile([128, 512], f32, name="ot", tag="ot")
                        # alternate DVE/ACT so the last group's four scales
                        # drain on two engines instead of serializing on DVE
                        if mt % 2 == 0:
                            nc.vector.tensor_scalar_mul(ot[:], pd[mt][:], rwt[:, mt:mt + 1])
                        else:
                            nc.scalar.mul(ot[:], pd[mt][:], rwt[:, mt:mt + 1])
                        # spread stores over three rings: the HBM write
                        # receipt (~2 us) serializes completions per ring,
                        # which otherwise dominates the kernel tail
                        eng = (nc.sync, nc.scalar, nc.gpsimd)[(nh * NT + mt) % 3]
                        eng.dma_start(dout[nh, mt], ot[:])

    nc.compile()
    return nc


@functools.lru_cache(maxsize=2)
def _get_exec(nb1: int = 0):
    """Compile the Bass program and return (nc, run_fn) with a cached jit.

    run_fn(in_maps) -> list of per-core {"d": np.ndarray}. Mirrors
    bass2jax.run_bass_via_pjrt's multi-core branch, but keeps the jitted
    function alive across kernel() calls so repeat invocations skip XLA
    and NEFF compilation.
    """
    import jax
    import concourse.mybir as mybir
    from concourse import bass2jax

    nc = _build(nb1)
    bass2jax.install_neuronx_cc_hook()

    partition_name = nc.partition_id_tensor.name if nc.partition_id_tensor else None
    in_names, out_names, out_avals = [], [], []
    zero_out_shapes = []
    for alloc in nc.m.functions[0].allocations:
        if not isinstance(alloc, mybir.MemoryLocationSet):
            continue
        name = alloc.memorylocations[0].name
        if alloc.kind == "ExternalInput":
            if name != partition_name:
                in_names.append(name)
        elif alloc.kind == "ExternalOutput":
            shape = tuple(alloc.tensor_shape)
            dtype = mybir.dt.np(alloc.dtype)
            out_names.append(name)
            out_avals.append(jax.core.ShapedArray(shape, dtype))
            zero_out_shapes.append((shape, dtype))
    n_params = len(in_names)
    n_outs = len(out_names)
    all_names = list(in_names) + list(out_names)
    if partition_name is not None:
        all_names.append(partition_name)
    donate = tuple(range(n_params, n_params + n_outs))

    def _body(*args):
        operands = list(args)
        if partition_name is not None:
            operands.append(bass2jax.partition_id_tensor())
        outs = bass2jax._bass_exec_p.bind(
            *operands,
            out_avals=tuple(out_avals),
            in_names=tuple(all_names),
            out_names=tuple(out_names),
            lowering_input_output_aliases=(),
            sim_require_finite=True,
            sim_require_nnan=True,
            nc=nc,
        )
        return tuple(outs)

    devices = jax.devices()[:NCORES]
    assert len(devices) == NCORES, f"need {NCORES} devices, have {len(jax.devices())}"
    mesh = bass2jax.Mesh(np.asarray(devices), ("core",))
    in_specs = (bass2jax.PartitionSpec("core"),) * (n_params + n_outs)
    out_specs = (bass2jax.PartitionSpec("core"),) * n_outs
    sharded = jax.jit(
        bass2jax.shard_map(
            _body, mesh=mesh, in_specs=in_specs, out_specs=out_specs, check_rep=False
        ),
        donate_argnums=donate,
        keep_unused=True,
    )

    def run_fn(in_maps):
        concat_in = [
            np.concatenate([np.asarray(m[name]) for m in in_maps], axis=0)
            for name in in_names
        ]
        zeros = [
            np.zeros((shape[0] * NCORES,) + shape[1:], dtype)
            for shape, dtype in zero_out_shapes
        ]
        out_arrs = sharded(*concat_in, *zeros)
        results = []
        for c in range(NCORES):
            res = {}
            for i, name in enumerate(out_names):
                arr = np.asarray(out_arrs[i])
                per = arr.shape[0] // NCORES
                res[name] = arr[c * per:(c + 1) * per]
            results.append(res)
        return results

    return nc, run_fn


def _route(x, w_router):
    """Top-2 routing: expert ids + softmax weights, matching jax.lax.top_k
    (descending, ties to the lower index) + jax.nn.softmax."""
    logits = x.astype(np.float64) @ w_router.astype(np.float64)
    top2 = np.argsort(-logits, axis=1, kind="stable")[:, :TOPK]
    vals = np.take_along_axis(logits, top2, 1).astype(np.float32)
    e = np.exp(vals - vals.max(axis=1, keepdims=True))
    w = (e / e.sum(axis=1, keepdims=True)).astype(np.float32)
    return top2, w


def _reference_numpy(x, w_router, w_gate, w_up, w_down):
    """Correct-but-slow dense fallback for shapes the device program doesn't cover."""
    x = x.astype(np.float32)
    logits = x @ w_router.astype(np.float32)
    n_exp = w_gate.shape[0]
    k = min(TOPK, n_exp)
    top = np.argsort(-logits, axis=1, kind="stable")[:, :k]
    vals = np.take_along_axis(logits, top, 1)
    ex = np.exp(vals - vals.max(1, keepdims=True))
    ww = (ex / ex.sum(1, keepdims=True)).astype(np.float32)
    w_dense = np.zeros_like(logits)
    t_ids = np.arange(x.shape[0])[:, None]
    w_dense[t_ids, top] = ww
    out = np.zeros((x.shape[0], w_down.shape[-1]), np.float32)
    for e in range(n_exp):
        g = x @ w_gate[e]
        u = x @ w_up[e]
        a = (g / (1.0 + np.exp(-g))) * u
        out += w_dense[:, e:e + 1] * (a @ w_down[e])
    return out


def _pack_core_inputs(x, wg_e, wu_e, wd_e, toks, ws, nb1=0):
    """Build one core's input map. Everything fed to the PE is converted
    to bf16 and laid out partition-major so every weight half is one
    contiguous 1 MB DMA ([128, 8 k-tiles, 512] with 8 KB rows) and x is
    four contiguous 512 KB quarters."""
    assert nb1 == 0
    n_e = len(toks)
    xg = np.zeros((B0, H), np.float32)
    xg[:n_e] = x[toks]
    # xT0[p, k, t] = xg[t, k*128+p]
    xT = np.ascontiguousarray(xg.T).reshape(KT, 128, B0).astype(BF16)
    xTq = np.ascontiguousarray(xT.transpose(1, 0, 2))
    rfull = np.zeros(B0, np.float32)
    rfull[:n_e] = ws
    # wgu[mc, half, p, kk, 0:256 | 256:512] = gate|up[(half*8+kk)*128+p, mc-chunk]
    wgu = np.empty((NMC, 2, 128, WHK, 2 * CHUNK), BF16)
    wgu[..., :CHUNK] = (
        wg_e.reshape(2, WHK, 128, NMC, CHUNK).transpose(3, 0, 2, 1, 4).astype(BF16)
    )
    wgu[..., CHUNK:] = (
        wu_e.reshape(2, WHK, 128, NMC, CHUNK).transpose(3, 0, 2, 1, 4).astype(BF16)
    )
    # wd[nh, half, p, kk, h] = w_down[(half*8+kk)*128+p, nh*512+h]
    wdt = np.ascontiguousarray(
        wd_e.reshape(KI // WHK, WHK, 128, NH, 512).transpose(3, 0, 2, 1, 4)
    ).astype(BF16)
    return {
        "xT0": xTq,
        "wgu": wgu,
        "wd": wdt,
        "rw": np.ascontiguousarray(rfull.reshape(NT, 128).T),
    }


def kernel(x, w_router, w_gate, w_up, w_down):
    x = np.ascontiguousarray(np.asarray(x, dtype=np.float32))
    w_router = np.asarray(w_router, dtype=np.float32)
    w_gate = np.ascontiguousarray(np.asarray(w_gate, dtype=np.float32))
    w_up = np.ascontiguousarray(np.asarray(w_up, dtype=np.float32))
    w_down = np.ascontiguousarray(np.asarray(w_down, dtype=np.float32))

    if (x.shape != (T, H) or w_router.shape != (H, E)
            or w_gate.shape != (E, H, II) or w_up.shape != (E, H, II)
            or w_down.shape != (E, II, H)):
        return _reference_numpy(x, w_router, w_gate, w_up, w_down)

    top2, w = _route(x, w_router)
    tok = np.repeat(np.arange(T), TOPK)
    te = top2.ravel()
    tw = w.ravel()
    toks_e, ws_e = [], []
    for e in range(E):
        sel = te == e
        toks_e.append(tok[sel])
        ws_e.append(tw[sel].astype(np.float32))

    # Capacity-factor dispatch: the device program handles up to B0=512
    # tokens per expert (98.5% of routed tokens for balanced routing); the
    # rare spill beyond capacity goes through an exact fp32 host path.
    nc, run_fn = _get_exec(0)

    in_maps = [
        _pack_core_inputs(x, w_gate[e], w_up[e], w_down[e],
                          toks_e[e][:B0], ws_e[e][:B0], 0)
        for e in range(E)
    ]

    try:
        results = run_fn(in_maps)
    except Exception:
        import time as _time
        _time.sleep(20)
        results = run_fn(in_maps)

    out = np.zeros((T, H), np.float32)
    for e in range(E):
        n_e = min(len(toks_e[e]), B0)
        # d: [NH, NT, 128, 512] -> [NT*128 tokens, NH*512 hidden]
        d = results[e]["d"].transpose(1, 2, 0, 3).reshape(B0, H)
        out[toks_e[e][:B0]] += d[:n_e]
        spill = toks_e[e][B0:]
        if spill.size:
            xe = x[spill]
            g = xe @ w_gate[e]
            u = xe @ w_up[e]
            a = (g / (1.0 + np.exp(-g))) * u
            out[spill] += (a @ w_down[e]) * ws_e[e][B0:, None]
    return out


# revision 18
# speedup vs baseline: 1.0143x; 1.0143x over previous
"""Trainium2 Bass kernel for nn_BlockSparseMLP (MoE gated MLP, E=8, top-2).

Strategy: expert parallelism over 8 NeuronCores. The router matmul
(x @ w_router, 67 MFLOP out of the 206 GFLOP total) plus the top-2
dispatch/gather and the final scatter-add combine run on the host; each
core runs the full gated MLP (gate/up, silu*up, down, weighted by the
routing prob) for the tokens routed to its expert.

All matmul operands are bf16 (host-converted; PSUM accumulation stays
fp32): full PE rate (1 row/cycle) like fp32r, half the HBM traffic.
Weights are streamed as contiguous 1 MB transfers ([128 part, 8 k-tiles,
512] halves, packed on the host so partition-major rows are 8 KB runs) —
128 KB tile-at-a-time DMA only sustains ~75 GB/s/queue and starved the
PE at startup; 1 MB transfers run at ~340 GB/s and keep the weight
stream ahead of the PE for the whole kernel.

Per-core device layout (capacity C = 512 tokens):
  phase 1 (gate/up): per I-chunk of 256, two 1 MB weight halves
    (gate|up packed side by side); weights stationary, xT moving
    (N=512). silu(gate)*up fused on ACT+DVE into aT ([I, C] bf16,
    SBUF-resident).
  phase 2 (down): stream w_down as 1 MB halves (moving [128,512]
    slices), aT tiles stationary, accumulate over I into [tokens, 512]
    psum tiles, scale by the routing weight on DVE, contiguous 256 KB
    fp32 stores on the HWDGE queues.
"""

import sys
import functools

sys.path.insert(0, "/opt/trn_rl_repo")

import numpy as np
import ml_dtypes

BF16 = ml_dtypes.bfloat16

T, H, II, E, TOPK = 2048, 2048, 4096, 8, 2
NCORES = 8
B0 = 512        # per-expert token capacity (moving N)
CHUNK = 256     # phase-1 I-chunk width
KT = H // 128   # 16 contraction tiles for gate/up
MTI = II // 128  # 32 I tiles
NMC = II // CHUNK  # 16 phase-1 chunks
JJ = CHUNK // 128  # 2 m-tiles per chunk
KI = II // 128  # 32 contraction tiles for down
NH = H // 512   # 4 output column chunks
NT = B0 // 128  # 4 token tiles
XQ = 4          # x is loaded as 4 quarter tiles of 4 k-slices each
WHK = 8         # k-tiles per 1 MB weight half


@functools.lru_cache(maxsize=2)
def _build(nb1: int = 0):
    """Build the SPMD Bass program (capacity B0 tokens; nb1 kept for
    test.py signature compat and must be 0 — overflow spills to host)."""
    assert nb1 == 0
    import concourse.mybir as mybir
    import concourse.tile as tile
    from concourse import bacc

    f32 = mybir.dt.float32
    bf16 = mybir.dt.bfloat16

    nc = bacc.Bacc(None)
    xT0 = nc.declare_dram_parameter("xT0", [128, KT, B0], bf16, isOutput=False)
    wgu = nc.declare_dram_parameter("wgu", [NMC, 2, 128, WHK, 2 * CHUNK], bf16, isOutput=False)
    wd = nc.declare_dram_parameter("wd", [NH, KI // WHK, 128, WHK, 512], bf16, isOutput=False)
    rw = nc.declare_dram_parameter("rw", [128, NT], f32, isOutput=False)
    dout = nc.declare_dram_parameter("d", [NH, NT, 128, 512], f32, isOutput=True)

    SILU = mybir.ActivationFunctionType.Silu

    with tile.TileContext(nc) as tc:
        with (
            tc.tile_pool(name="pers", bufs=1) as pers,
            tc.tile_pool(name="wpool", bufs=8) as wpool,
            tc.tile_pool(name="wdpool", bufs=4) as wdpool,
        ):
            aT0 = pers.tile([128, MTI, B0], bf16)
            rwt = pers.tile([128, NT], f32)

            with (
                tc.tile_pool(name="xp", bufs=1) as xp,
                tc.tile_pool(name="ps1", bufs=1, space="PSUM") as ps1,
                tc.tile_pool(name="pw", bufs=1, space="PSUM") as pw,
                tc.tile_pool(name="sp", bufs=2) as sp,
            ):
                # PE warm-up: DMA delivery crawls for the first ~5 us after
                # issue (cold HBM + all 8 cores bursting at once), and the
                # HAM clock gate holds the PE at 1.2 GHz until it has been
                # busy for a ~3.4 us activity window. Spend the unavoidable
                # DMA wait on zero matmuls into a scratch PSUM bank so the
                # clock is already 2.4 GHz when the first real matmul issues.
                wz = xp.tile([128, 128], bf16, name="wz")
                nc.vector.memset(wz[:], 0.0)
                pwarm = pw.tile([128, 64], f32, name="pwarm")
                for _ in range(64):
                    nc.tensor.matmul(pwarm[:64, :], wz[:, :64], wz[:, :64],
                                     start=True, stop=True)

                # Startup choreography: xq0 leads the sync HWDGE ring so the
                # first matmul's moving operand lands first; the remaining x
                # quarters ride the gpsimd (SWDGE) ring so no weight half
                # queues behind 2 MB of x. Chunk 0's weights arrive as
                # 256 KB quarter-tiles; later chunks stream as 1 MB halves
                # alternating scalar/sync.
                xq = []
                for q in range(XQ):
                    xt = xp.tile([128, KT // XQ, B0], bf16, name=f"xq{q}")
                    eng = nc.sync if q == 0 else nc.gpsimd
                    eng.dma_start(xt[:], xT0[:, q * 4:q * 4 + 4, :])
                    xq.append(xt)

                def xk_of(k):
                    return xq[k // 4][:, k % 4, :]

                for mc in range(NMC):
                    if mc == 0:
                        wq = []
                        for i in range(8):
                            wt = xp.tile([128, 2, 2 * CHUNK], bf16, name=f"wq{i}")
                            eng = nc.scalar if i % 2 == 0 else nc.sync
                            h, o = divmod(i * 2, WHK)
                            eng.dma_start(wt[:], wgu[0, h][:, o:o + 2, :])
                            wq.append(wt)

                        def wk_of(k):
                            return wq[k // 2][:, k % 2, :]
                    else:
                        wb = []
                        for h in range(2):
                            hidx = mc * 2 + h
                            eng = nc.scalar if hidx % 2 == 0 else nc.sync
                            wbh = wpool.tile([128, WHK, 2 * CHUNK], bf16, name="wbh", tag="w")
                            eng.dma_start(wbh[:], wgu[mc, h])
                            wb.append(wbh)
                        wk_of = lambda k, wb=wb: wb[k // WHK][:, k % WHK, :]

                    pg0 = [ps1.tile([128, B0], f32, name="pg0", tag="pg0", bufs=3)
                           for _ in range(JJ)]
                    pu0 = [ps1.tile([128, B0], f32, name="pu0", tag="pu0", bufs=3)
                           for _ in range(JJ)]

                    for k in range(KT):
                        st = dict(start=(k == 0), stop=(k == KT - 1))
                        wk = wk_of(k)
                        xk = xk_of(k)
                        for j in range(JJ):
                            nc.tensor.matmul(
                                pg0[j][:], wk[:, j * 128:(j + 1) * 128], xk, **st
                            )
                        for j in range(JJ):
                            nc.tensor.matmul(
                                pu0[j][:],
                                wk[:, CHUNK + j * 128:CHUNK + (j + 1) * 128],
                                xk, **st
                            )

                    for j in range(JJ):
                        m = mc * JJ + j
                        sg = sp.tile([128, B0], f32, name="sg", tag="sg")
                        nc.scalar.activation(sg[:], pg0[j][:], SILU)
                        nc.vector.tensor_mul(aT0[:, m, :], sg[:], pu0[j][:])

            with (
                tc.tile_pool(name="ps2", bufs=1, space="PSUM") as ps2,
                tc.tile_pool(name="op", bufs=6) as op,
            ):
                nc.gpsimd.dma_start(rwt[:], rw[:])
                for nh in range(NH):
                    pd = [ps2.tile([128, 512], f32, name="pd", tag="pd", bufs=7)
                          for _ in range(NT)]
                    for h in range(KI // WHK):
                        hidx = nh * (KI // WHK) + h
                        # dedicated pool: the first 4 halves have no slot
                        # dependency on phase-1 tiles, so they prefetch during
                        # phase 1 and the phase transition never stalls. sync
                        # leads — scalar is busy with the last silu/mul work.
                        eng = nc.sync if hidx % 2 == 0 else nc.scalar
                        wdh = wdpool.tile([128, WHK, 512], bf16, name="wdh", tag="wd")
                        eng.dma_start(wdh[:], wd[nh, h])
                        for kk in range(WHK):
                            k = h * WHK + kk
                            st = dict(start=(k == 0), stop=(k == KI - 1))
                            for mt in range(NT):
                                nc.tensor.matmul(
                                    pd[mt][:], aT0[:, k, mt * 128:(mt + 1) * 128],
                                    wdh[:, kk, :], **st
                                )
                    for mt in range(NT):
                        ot = op.tile([128, 512], f32, name="ot", tag="ot")
                        # alternate DVE/ACT so the last group's four scales
                        # drain on two engines instead of serializing on DVE
                        if mt % 2 == 0:
                            nc.vector.tensor_scalar_mul(ot[:], pd[mt][:], rwt[:, mt:mt + 1])
                        else:
                            nc.scalar.mul(ot[:], pd[mt][:], rwt[:, mt:mt + 1])
                        # spread stores over three rings: the HBM write
                        # receipt (~2 us) serializes completions per ring,
                        # which otherwise dominates the kernel tail
                        eng = (nc.sync, nc.scalar, nc.gpsimd)[(nh * NT + mt) % 3]
                        eng.dma_start(dout[nh, mt], ot[:])

    nc.compile()
    return nc


@functools.lru_cache(maxsize=2)
def _get_exec(nb1: int = 0):
    """Compile the Bass program and return (nc, run_fn) with a cached jit.

    run_fn(in_maps) -> list of per-core {"d": np.ndarray}. Mirrors
    bass2jax.run_bass_via_pjrt's multi-core branch, but keeps the jitted
    function alive across kernel() calls so repeat invocations skip XLA
    and NEFF compilation.
    """
    import jax
    import concourse.mybir as mybir
    from concourse import bass2jax

    nc = _build(nb1)
    bass2jax.install_neuronx_cc_hook()

    partition_name = nc.partition_id_tensor.name if nc.partition_id_tensor else None
    in_names, out_names, out_avals = [], [], []
    zero_out_shapes = []
    for alloc in nc.m.functions[0].allocations:
        if not isinstance(alloc, mybir.MemoryLocationSet):
            continue
        name = alloc.memorylocations[0].name
        if alloc.kind == "ExternalInput":
            if name != partition_name:
                in_names.append(name)
        elif alloc.kind == "ExternalOutput":
            shape = tuple(alloc.tensor_shape)
            dtype = mybir.dt.np(alloc.dtype)
            out_names.append(name)
            out_avals.append(jax.core.ShapedArray(shape, dtype))
            zero_out_shapes.append((shape, dtype))
    n_params = len(in_names)
    n_outs = len(out_names)
    all_names = list(in_names) + list(out_names)
    if partition_name is not None:
        all_names.append(partition_name)
    donate = tuple(range(n_params, n_params + n_outs))

    def _body(*args):
        operands = list(args)
        if partition_name is not None:
            operands.append(bass2jax.partition_id_tensor())
        outs = bass2jax._bass_exec_p.bind(
            *operands,
            out_avals=tuple(out_avals),
            in_names=tuple(all_names),
            out_names=tuple(out_names),
            lowering_input_output_aliases=(),
            sim_require_finite=True,
            sim_require_nnan=True,
            nc=nc,
        )
        return tuple(outs)

    devices = jax.devices()[:NCORES]
    assert len(devices) == NCORES, f"need {NCORES} devices, have {len(jax.devices())}"
    mesh = bass2jax.Mesh(np.asarray(devices), ("core",))
    in_specs = (bass2jax.PartitionSpec("core"),) * (n_params + n_outs)
    out_specs = (bass2jax.PartitionSpec("core"),) * n_outs
    sharded = jax.jit(
        bass2jax.shard_map(
            _body, mesh=mesh, in_specs=in_specs, out_specs=out_specs, check_rep=False
        ),
        donate_argnums=donate,
        keep_unused=True,
    )

    def run_fn(in_maps):
        concat_in = [
            np.concatenate([np.asarray(m[name]) for m in in_maps], axis=0)
            for name in in_names
        ]
        zeros = [
            np.zeros((shape[0] * NCORES,) + shape[1:], dtype)
            for shape, dtype in zero_out_shapes
        ]
        out_arrs = sharded(*concat_in, *zeros)
        results = []
        for c in range(NCORES):
            res = {}
            for i, name in enumerate(out_names):
                arr = np.asarray(out_arrs[i])
                per = arr.shape[0] // NCORES
                res[name] = arr[c * per:(c + 1) * per]
            results.append(res)
        return results

    return nc, run_fn


def _route(x, w_router):
    """Top-2 routing: expert ids + softmax weights, matching jax.lax.top_k
    (descending, ties to the lower index) + jax.nn.softmax."""
    logits = x.astype(np.float64) @ w_router.astype(np.float64)
    top2 = np.argsort(-logits, axis=1, kind="stable")[:, :TOPK]
    vals = np.take_along_axis(logits, top2, 1).astype(np.float32)
    e = np.exp(vals - vals.max(axis=1, keepdims=True))
    w = (e / e.sum(axis=1, keepdims=True)).astype(np.float32)
    return top2, w


def _reference_numpy(x, w_router, w_gate, w_up, w_down):
    """Correct-but-slow dense fallback for shapes the device program doesn't cover."""
    x = x.astype(np.float32)
    logits = x @ w_router.astype(np.float32)
    n_exp = w_gate.shape[0]
    k = min(TOPK, n_exp)
    top = np.argsort(-logits, axis=1, kind="stable")[:, :k]
    vals = np.take_along_axis(logits, top, 1)
    ex = np.exp(vals - vals.max(1, keepdims=True))
    ww = (ex / ex.sum(1, keepdims=True)).astype(np.float32)
    w_dense = np.zeros_like(logits)
    t_ids = np.arange(x.shape[0])[:, None]
    w_dense[t_ids, top] = ww
    out = np.zeros((x.shape[0], w_down.shape[-1]), np.float32)
    for e in range(n_exp):
        g = x @ w_gate[e]
        u = x @ w_up[e]
        a = (g / (1.0 + np.exp(-g))) * u
        out += w_dense[:, e:e + 1] * (a @ w_down[e])
    return out


def _pack_core_inputs(x, wg_e, wu_e, wd_e, toks, ws, nb1=0):
    """Build one core's input map. Everything fed to the PE is converted
    to bf16 and laid out partition-major so every weight half is one
    contiguous 1 MB DMA ([128, 8 k-tiles, 512] with 8 KB rows) and x is
    four contiguous 512 KB quarters."""
    assert nb1 == 0
    n_e = len(toks)
    xg = np.zeros((B0, H), np.float32)
    xg[:n_e] = x[toks]
    # xT0[p, k, t] = xg[t, k*128+p]
    xT = np.ascontiguousarray(xg.T).reshape(KT, 128, B0).astype(BF16)
    xTq = np.ascontiguousarray(xT.transpose(1, 0, 2))
    rfull = np.zeros(B0, np.float32)
    rfull[:n_e] = ws
    # wgu[mc, half, p, kk, 0:256 | 256:512] = gate|up[(half*8+kk)*128+p, mc-chunk]
    wgu = np.empty((NMC, 2, 128, WHK, 2 * CHUNK), BF16)
    wgu[..., :CHUNK] = (
        wg_e.reshape(2, WHK, 128, NMC, CHUNK).transpose(3, 0, 2, 1, 4).astype(BF16)
    )
    wgu[..., CHUNK:] = (
        wu_e.reshape(2, WHK, 128, NMC, CHUNK).transpose(3, 0, 2, 1, 4).astype(BF16)
    )
    # wd[nh, half, p, kk, h] = w_down[(half*8+kk)*128+p, nh*512+h]
    wdt = np.ascontiguousarray(
        wd_e.reshape(KI // WHK, WHK, 128, NH, 512).transpose(3, 0, 2, 1, 4)
    ).astype(BF16)
    return {
        "xT0": xTq,
        "wgu": wgu,
        "wd": wdt,
        "rw": np.ascontiguousarray(rfull.reshape(NT, 128).T),
    }


def kernel(x, w_router, w_gate, w_up, w_down):
    x = np.ascontiguousarray(np.asarray(x, dtype=np.float32))
    w_router = np.asarray(w_router, dtype=np.float32)
    w_gate = np.ascontiguousarray(np.asarray(w_gate, dtype=np.float32))
    w_up = np.ascontiguousarray(np.asarray(w_up, dtype=np.float32))
    w_down = np.ascontiguousarray(np.asarray(w_down, dtype=np.float32))

    if (x.shape != (T, H) or w_router.shape != (H, E)
            or w_gate.shape != (E, H, II) or w_up.shape != (E, H, II)
            or w_down.shape != (E, II, H)):
        return _reference_numpy(x, w_router, w_gate, w_up, w_down)

    top2, w = _route(x, w_router)
    tok = np.repeat(np.arange(T), TOPK)
    te = top2.ravel()
    tw = w.ravel()
    toks_e, ws_e = [], []
    for e in range(E):
        sel = te == e
        toks_e.append(tok[sel])
        ws_e.append(tw[sel].astype(np.float32))

    # Capacity-factor dispatch: the device program handles up to B0=512
    # tokens per expert (98.5% of routed tokens for balanced routing); the
    # rare spill beyond capacity goes through an exact fp32 host path.
    nc, run_fn = _get_exec(0)

    in_maps = [
        _pack_core_inputs(x, w_gate[e], w_up[e], w_down[e],
                          toks_e[e][:B0], ws_e[e][:B0], 0)
        for e in range(E)
    ]

    try:
        results = run_fn(in_maps)
    except Exception:
        import time as _time
        _time.sleep(20)
        results = run_fn(in_maps)

    out = np.zeros((T, H), np.float32)
    for e in range(E):
        n_e = min(len(toks_e[e]), B0)
        # d: [NH, NT, 128, 512] -> [NT*128 tokens, NH*512 hidden]
        d = results[e]["d"].transpose(1, 2, 0, 3).reshape(B0, H)
        out[toks_e[e][:B0]] += d[:n_e]
        spill = toks_e[e][B0:]
        if spill.size:
            xe = x[spill]
            g = xe @ w_gate[e]
            u = xe @ w_up[e]
            a = (g / (1.0 + np.exp(-g))) * u
            out[spill] += (a @ w_down[e]) * ws_e[e][B0:, None]
    return out


# revision 23
# speedup vs baseline: 1.0261x; 1.0116x over previous
"""Trainium2 Bass kernel for nn_BlockSparseMLP (MoE gated MLP, E=8, top-2).

Strategy: expert parallelism over 8 NeuronCores. The router matmul
(x @ w_router, 67 MFLOP out of the 206 GFLOP total) plus the top-2
dispatch/gather and the final scatter-add combine run on the host; each
core runs the full gated MLP (gate/up, silu*up, down, weighted by the
routing prob) for the tokens routed to its expert.

All matmul operands are bf16 (host-converted; PSUM accumulation stays
fp32): full PE rate (1 row/cycle) like fp32r, half the HBM traffic.
Weights are streamed as contiguous 1 MB transfers ([128 part, 8 k-tiles,
512] halves, packed on the host so partition-major rows are 8 KB runs) —
128 KB tile-at-a-time DMA only sustains ~75 GB/s/queue and starved the
PE at startup; 1 MB transfers run at ~340 GB/s and keep the weight
stream ahead of the PE for the whole kernel.

Per-core device layout (capacity C = 512 tokens):
  phase 1 (gate/up): per I-chunk of 256, two 1 MB weight halves
    (gate|up packed side by side); weights stationary, xT moving
    (N=512). silu(gate)*up fused on ACT+DVE into aT ([I, C] bf16,
    SBUF-resident).
  phase 2 (down): stream w_down as 1 MB halves (moving [128,512]
    slices), aT tiles stationary, accumulate over I into [tokens, 512]
    psum tiles, scale by the routing weight on DVE, contiguous 256 KB
    fp32 stores on the HWDGE queues.
"""

import sys
import functools

sys.path.insert(0, "/opt/trn_rl_repo")

import numpy as np
import ml_dtypes

BF16 = ml_dtypes.bfloat16

T, H, II, E, TOPK = 2048, 2048, 4096, 8, 2
NCORES = 8
B0 = 512        # per-expert token capacity (moving N)
CHUNK = 256     # phase-1 I-chunk width
KT = H // 128   # 16 contraction tiles for gate/up
MTI = II // 128  # 32 I tiles
NMC = II // CHUNK  # 16 phase-1 chunks
JJ = CHUNK // 128  # 2 m-tiles per chunk
KI = II // 128  # 32 contraction tiles for down
NH = H // 512   # 4 output column chunks
NT = B0 // 128  # 4 token tiles
XQ = 4          # x is loaded as 4 quarter tiles of 4 k-slices each
WHK = 8         # k-tiles per 1 MB weight half


@functools.lru_cache(maxsize=2)
def _build(nb1: int = 0):
    """Build the SPMD Bass program (capacity B0 tokens; nb1 kept for
    test.py signature compat and must be 0 — overflow spills to host)."""
    assert nb1 == 0
    import concourse.mybir as mybir
    import concourse.tile as tile
    from concourse import bacc

    f32 = mybir.dt.float32
    bf16 = mybir.dt.bfloat16

    nc = bacc.Bacc(None)
    xT0 = nc.declare_dram_parameter("xT0", [128, KT, B0], bf16, isOutput=False)
    wgu = nc.declare_dram_parameter("wgu", [NMC, 2, 128, WHK, 2 * CHUNK], bf16, isOutput=False)
    wd = nc.declare_dram_parameter("wd", [NH, KI // WHK, 128, WHK, 512], bf16, isOutput=False)
    rw = nc.declare_dram_parameter("rw", [128, NT], f32, isOutput=False)
    dout = nc.declare_dram_parameter("d", [NH, NT, 128, 512], f32, isOutput=True)

    SILU = mybir.ActivationFunctionType.Silu

    with tile.TileContext(nc) as tc:
        with (
            tc.tile_pool(name="pers", bufs=1) as pers,
            tc.tile_pool(name="wpool", bufs=8) as wpool,
            tc.tile_pool(name="wdpool", bufs=4) as wdpool,
            # One PSUM pool spans both phases: phase 2's pd tiles reuse the
            # pg0/pu0 tag slots, whose rotation frees before phase 1 ends,
            # so the phase transition has no pool-close stall on the PE.
            tc.tile_pool(name="psp", bufs=1, space="PSUM") as psp,
        ):
            aT0 = pers.tile([128, MTI, B0], bf16)
            rwt = pers.tile([128, NT], f32)

            with (
                tc.tile_pool(name="xp", bufs=1) as xp,
                tc.tile_pool(name="sp", bufs=2) as sp,
            ):
                # PE warm-up: DMA delivery crawls for the first ~5 us after
                # issue (cold HBM + all 8 cores bursting at once), and the
                # HAM clock gate holds the PE at 1.2 GHz until it has been
                # busy for a ~3.4 us activity window. Spend the unavoidable
                # DMA wait on zero matmuls into a scratch PSUM slot so the
                # clock is already 2.4 GHz when the first real matmul issues.
                wz = xp.tile([128, 128], bf16, name="wz")
                nc.vector.memset(wz[:], 0.0)
                pwarm = psp.tile([128, 64], f32, name="pwarm", tag="pg0", bufs=3)
                for _ in range(128):
                    nc.tensor.matmul(pwarm[:64, :], wz[:, :64], wz[:, :64],
                                     start=True, stop=True)

                # Startup choreography: xq0 leads the sync HWDGE ring so the
                # first matmul's moving operand lands first; the remaining x
                # quarters ride the gpsimd (SWDGE) ring so no weight half
                # queues behind 2 MB of x. Chunk 0's weights arrive as
                # 256 KB quarter-tiles; later chunks stream as 1 MB halves
                # alternating scalar/sync.
                xq = []
                for q in range(XQ):
                    xt = xp.tile([128, KT // XQ, B0], bf16, name=f"xq{q}")
                    eng = nc.sync if q == 0 else nc.gpsimd
                    eng.dma_start(xt[:], xT0[:, q * 4:q * 4 + 4, :])
                    xq.append(xt)

                def xk_of(k):
                    return xq[k // 4][:, k % 4, :]

                for mc in range(NMC):
                    if mc == 0:
                        wq = []
                        for i in range(8):
                            wt = xp.tile([128, 2, 2 * CHUNK], bf16, name=f"wq{i}")
                            eng = nc.scalar if i % 2 == 0 else nc.sync
                            h, o = divmod(i * 2, WHK)
                            eng.dma_start(wt[:], wgu[0, h][:, o:o + 2, :])
                            wq.append(wt)

                        def wk_of(k):
                            return wq[k // 2][:, k % 2, :]
                    else:
                        wb = []
                        for h in range(2):
                            hidx = mc * 2 + h
                            eng = nc.scalar if hidx % 2 == 0 else nc.sync
                            wbh = wpool.tile([128, WHK, 2 * CHUNK], bf16, name="wbh", tag="w")
                            eng.dma_start(wbh[:], wgu[mc, h])
                            wb.append(wbh)
                        wk_of = lambda k, wb=wb: wb[k // WHK][:, k % WHK, :]

                    pg0 = [psp.tile([128, B0], f32, name="pg0", tag="pg0", bufs=3)
                           for _ in range(JJ)]
                    pu0 = [psp.tile([128, B0], f32, name="pu0", tag="pu0", bufs=3)
                           for _ in range(JJ)]

                    for k in range(KT):
                        st = dict(start=(k == 0), stop=(k == KT - 1))
                        wk = wk_of(k)
                        xk = xk_of(k)
                        for j in range(JJ):
                            nc.tensor.matmul(
                                pg0[j][:], wk[:, j * 128:(j + 1) * 128], xk, **st
                            )
                        for j in range(JJ):
                            nc.tensor.matmul(
                                pu0[j][:],
                                wk[:, CHUNK + j * 128:CHUNK + (j + 1) * 128],
                                xk, **st
                            )

                    for j in range(JJ):
                        m = mc * JJ + j
                        sg = sp.tile([128, B0], f32, name="sg", tag="sg")
                        nc.scalar.activation(sg[:], pg0[j][:], SILU)
                        nc.vector.tensor_mul(aT0[:, m, :], sg[:], pu0[j][:])

            with tc.tile_pool(name="op", bufs=6) as op:
                nc.gpsimd.dma_start(rwt[:], rw[:])
                for nh in range(NH):
                    pd = [psp.tile([128, 512], f32, name="pd",
                                   tag=("pg0" if mt < 2 else "pu0"), bufs=3)
                          for mt in range(NT)]
                    for h in range(KI // WHK):
                        hidx = nh * (KI // WHK) + h
                        # dedicated pool: the first 4 halves have no slot
                        # dependency on phase-1 tiles, so they prefetch during
                        # phase 1 and the phase transition never stalls. sync
                        # leads — scalar is busy with the last silu/mul work.
                        eng = nc.sync if hidx % 2 == 0 else nc.scalar
                        wdh = wdpool.tile([128, WHK, 512], bf16, name="wdh", tag="wd")
                        eng.dma_start(wdh[:], wd[nh, h])
                        for kk in range(WHK):
                            k = h * WHK + kk
                            st = dict(start=(k == 0), stop=(k == KI - 1))
                            for mt in range(NT):
                                nc.tensor.matmul(
                                    pd[mt][:], aT0[:, k, mt * 128:(mt + 1) * 128],
                                    wdh[:, kk, :], **st
                                )
                    for mt in range(NT):
                        ot = op.tile([128, 512], f32, name="ot", tag="ot")
                        # alternate DVE/ACT so the last group's four scales
                        # drain on two engines instead of serializing on DVE
                        if mt % 2 == 0:
                            nc.vector.tensor_scalar_mul(ot[:], pd[mt][:], rwt[:, mt:mt + 1])
                        else:
                            nc.scalar.mul(ot[:], pd[mt][:], rwt[:, mt:mt + 1])
                        # spread stores over three rings: the HBM write
                        # receipt (~2 us) serializes completions per ring,
                        # which otherwise dominates the kernel tail. The last
                        # group stays on the low-latency HWDGE rings.
                        if nh == NH - 1:
                            eng = nc.sync if mt % 2 == 0 else nc.scalar
                        else:
                            eng = (nc.sync, nc.scalar, nc.gpsimd)[(nh * NT + mt) % 3]
                        eng.dma_start(dout[nh, mt], ot[:])

    nc.compile()
    return nc


@functools.lru_cache(maxsize=2)
def _get_exec(nb1: int = 0):
    """Compile the Bass program and return (nc, run_fn) with a cached jit.

    run_fn(in_maps) -> list of per-core {"d": np.ndarray}. Mirrors
    bass2jax.run_bass_via_pjrt's multi-core branch, but keeps the jitted
    function alive across kernel() calls so repeat invocations skip XLA
    and NEFF compilation.
    """
    import jax
    import concourse.mybir as mybir
    from concourse import bass2jax

    nc = _build(nb1)
    bass2jax.install_neuronx_cc_hook()

    partition_name = nc.partition_id_tensor.name if nc.partition_id_tensor else None
    in_names, out_names, out_avals = [], [], []
    zero_out_shapes = []
    for alloc in nc.m.functions[0].allocations:
        if not isinstance(alloc, mybir.MemoryLocationSet):
            continue
        name = alloc.memorylocations[0].name
        if alloc.kind == "ExternalInput":
            if name != partition_name:
                in_names.append(name)
        elif alloc.kind == "ExternalOutput":
            shape = tuple(alloc.tensor_shape)
            dtype = mybir.dt.np(alloc.dtype)
            out_names.append(name)
            out_avals.append(jax.core.ShapedArray(shape, dtype))
            zero_out_shapes.append((shape, dtype))
    n_params = len(in_names)
    n_outs = len(out_names)
    all_names = list(in_names) + list(out_names)
    if partition_name is not None:
        all_names.append(partition_name)
    donate = tuple(range(n_params, n_params + n_outs))

    def _body(*args):
        operands = list(args)
        if partition_name is not None:
            operands.append(bass2jax.partition_id_tensor())
        outs = bass2jax._bass_exec_p.bind(
            *operands,
            out_avals=tuple(out_avals),
            in_names=tuple(all_names),
            out_names=tuple(out_names),
            lowering_input_output_aliases=(),
            sim_require_finite=True,
            sim_require_nnan=True,
            nc=nc,
        )
        return tuple(outs)

    devices = jax.devices()[:NCORES]
    assert len(devices) == NCORES, f"need {NCORES} devices, have {len(jax.devices())}"
    mesh = bass2jax.Mesh(np.asarray(devices), ("core",))
    in_specs = (bass2jax.PartitionSpec("core"),) * (n_params + n_outs)
    out_specs = (bass2jax.PartitionSpec("core"),) * n_outs
    sharded = jax.jit(
        bass2jax.shard_map(
            _body, mesh=mesh, in_specs=in_specs, out_specs=out_specs, check_rep=False
        ),
        donate_argnums=donate,
        keep_unused=True,
    )

    def run_fn(in_maps):
        concat_in = [
            np.concatenate([np.asarray(m[name]) for m in in_maps], axis=0)
            for name in in_names
        ]
        zeros = [
            np.zeros((shape[0] * NCORES,) + shape[1:], dtype)
            for shape, dtype in zero_out_shapes
        ]
        out_arrs = sharded(*concat_in, *zeros)
        results = []
        for c in range(NCORES):
            res = {}
            for i, name in enumerate(out_names):
                arr = np.asarray(out_arrs[i])
                per = arr.shape[0] // NCORES
                res[name] = arr[c * per:(c + 1) * per]
            results.append(res)
        return results

    return nc, run_fn


def _route(x, w_router):
    """Top-2 routing: expert ids + softmax weights, matching jax.lax.top_k
    (descending, ties to the lower index) + jax.nn.softmax."""
    logits = x.astype(np.float64) @ w_router.astype(np.float64)
    top2 = np.argsort(-logits, axis=1, kind="stable")[:, :TOPK]
    vals = np.take_along_axis(logits, top2, 1).astype(np.float32)
    e = np.exp(vals - vals.max(axis=1, keepdims=True))
    w = (e / e.sum(axis=1, keepdims=True)).astype(np.float32)
    return top2, w


def _reference_numpy(x, w_router, w_gate, w_up, w_down):
    """Correct-but-slow dense fallback for shapes the device program doesn't cover."""
    x = x.astype(np.float32)
    logits = x @ w_router.astype(np.float32)
    n_exp = w_gate.shape[0]
    k = min(TOPK, n_exp)
    top = np.argsort(-logits, axis=1, kind="stable")[:, :k]
    vals = np.take_along_axis(logits, top, 1)
    ex = np.exp(vals - vals.max(1, keepdims=True))
    ww = (ex / ex.sum(1, keepdims=True)).astype(np.float32)
    w_dense = np.zeros_like(logits)
    t_ids = np.arange(x.shape[0])[:, None]
    w_dense[t_ids, top] = ww
    out = np.zeros((x.shape[0], w_down.shape[-1]), np.float32)
    for e in range(n_exp):
        g = x @ w_gate[e]
        u = x @ w_up[e]
        a = (g / (1.0 + np.exp(-g))) * u
        out += w_dense[:, e:e + 1] * (a @ w_down[e])
    return out


def _pack_core_inputs(x, wg_e, wu_e, wd_e, toks, ws, nb1=0):
    """Build one core's input map. Everything fed to the PE is converted
    to bf16 and laid out partition-major so every weight half is one
    contiguous 1 MB DMA ([128, 8 k-tiles, 512] with 8 KB rows) and x is
    four contiguous 512 KB quarters."""
    assert nb1 == 0
    n_e = len(toks)
    xg = np.zeros((B0, H), np.float32)
    xg[:n_e] = x[toks]
    # xT0[p, k, t] = xg[t, k*128+p]
    xT = np.ascontiguousarray(xg.T).reshape(KT, 128, B0).astype(BF16)
    xTq = np.ascontiguousarray(xT.transpose(1, 0, 2))
    rfull = np.zeros(B0, np.float32)
    rfull[:n_e] = ws
    # wgu[mc, half, p, kk, 0:256 | 256:512] = gate|up[(half*8+kk)*128+p, mc-chunk]
    wgu = np.empty((NMC, 2, 128, WHK, 2 * CHUNK), BF16)
    wgu[..., :CHUNK] = (
        wg_e.reshape(2, WHK, 128, NMC, CHUNK).transpose(3, 0, 2, 1, 4).astype(BF16)
    )
    wgu[..., CHUNK:] = (
        wu_e.reshape(2, WHK, 128, NMC, CHUNK).transpose(3, 0, 2, 1, 4).astype(BF16)
    )
    # wd[nh, half, p, kk, h] = w_down[(half*8+kk)*128+p, nh*512+h]
    wdt = np.ascontiguousarray(
        wd_e.reshape(KI // WHK, WHK, 128, NH, 512).transpose(3, 0, 2, 1, 4)
    ).astype(BF16)
    return {
        "xT0": xTq,
        "wgu": wgu,
        "wd": wdt,
        "rw": np.ascontiguousarray(rfull.reshape(NT, 128).T),
    }


def kernel(x, w_router, w_gate, w_up, w_down):
    x = np.ascontiguousarray(np.asarray(x, dtype=np.float32))
    w_router = np.asarray(w_router, dtype=np.float32)
    w_gate = np.ascontiguousarray(np.asarray(w_gate, dtype=np.float32))
    w_up = np.ascontiguousarray(np.asarray(w_up, dtype=np.float32))
    w_down = np.ascontiguousarray(np.asarray(w_down, dtype=np.float32))

    if (x.shape != (T, H) or w_router.shape != (H, E)
            or w_gate.shape != (E, H, II) or w_up.shape != (E, H, II)
            or w_down.shape != (E, II, H)):
        return _reference_numpy(x, w_router, w_gate, w_up, w_down)

    top2, w = _route(x, w_router)
    tok = np.repeat(np.arange(T), TOPK)
    te = top2.ravel()
    tw = w.ravel()
    toks_e, ws_e = [], []
    for e in range(E):
        sel = te == e
        toks_e.append(tok[sel])
        ws_e.append(tw[sel].astype(np.float32))

    # Capacity-factor dispatch: the device program handles up to B0=512
    # tokens per expert (98.5% of routed tokens for balanced routing); the
    # rare spill beyond capacity goes through an exact fp32 host path.
    nc, run_fn = _get_exec(0)

    in_maps = [
        _pack_core_inputs(x, w_gate[e], w_up[e], w_down[e],
                          toks_e[e][:B0], ws_e[e][:B0], 0)
        for e in range(E)
    ]

    try:
        results = run_fn(in_maps)
    except Exception:
        import time as _time
        _time.sleep(20)
        results = run_fn(in_maps)

    out = np.zeros((T, H), np.float32)
    for e in range(E):
        n_e = min(len(toks_e[e]), B0)
        # d: [NH, NT, 128, 512] -> [NT*128 tokens, NH*512 hidden]
        d = results[e]["d"].transpose(1, 2, 0, 3).reshape(B0, H)
        out[toks_e[e][:B0]] += d[:n_e]
        spill = toks_e[e][B0:]
        if spill.size:
            xe = x[spill]
            g = xe @ w_gate[e]
            u = xe @ w_up[e]
            a = (g / (1.0 + np.exp(-g))) * u
            out[spill] += (a @ w_down[e]) * ws_e[e][B0:, None]
    return out
